# revision 30
# baseline (speedup 1.0000x reference)
"""AttentionAggregator (GAT-style) Trainium2 Bass kernel, 8-core SPMD.

Math (see reference):
    node_t  = node @ W.T ; neigh_t = nbr @ W.T
    scores  = leaky_relu(node_t@a1 [i] + neigh_t@a2 [j], 0.2)  masked to the 10
              sampled neighbors per node (duplicate samples dropped),
    out     = softmax(scores) @ neigh_t

Key identities:
    node_t @ a1 = node @ (W.T @ a1) = node @ v1     (node_t never materialized)
    s2[j]       = neigh_t[j] @ a2                   (per neighbor-row scalar)

Sharding (8 cores):
    - neighbor rows (M=8192) sharded 1024/core for the big matmul
    - augmented rows [s2_f32 | neigh_t row (f16)] AllGathered in 4 chunks
      (layout [chunk, rank, 256, ROW]; gather indices host-remapped so each
      chunk ships as soon as its 2 i-tiles finish, overlapping the matmul)
    - nodes (N=4096) sharded 512/core for s1 + gather + softmax + aggregation

Measured (8x trn2 NeuronCores via axon):
    relative error vs fp32 reference: 2.1e-3 max-abs / 6.0e-4 l2
    device time per invocation: ~434 us (repeat-differential wall-clock;
    no NTFF profiler available on this axon shim)
"""

import numpy as np

N, M, K, O, S = 4096, 8192, 4096, 1024, 10
NCORES = 8
NB, MB = N // NCORES, M // NCORES  # 512 nodes, 1024 neighbor rows per core
NT = NB // 128                     # 4 node tiles per core
IT = MB // 128                     # 8 neighbor-row tiles per core
ROW = 1152                         # f16 elems per augmented row (2304 B, %256==0)
ROFF = 64                          # f16 offset of row data; s2 (f32) at elems 0:2
IDXW = (128 * S) // 16             # 80 int16 idx columns per node tile

TRACE = False
TRACE_KW = {}
LAST_RESULTS = None
_CACHE = {}
STAGE = 4    # debug: 1=W+v1, 2=+main matmul+s1, 3=+allgather, 4=full
SUBSTAGE = 3  # within stage 4: 1=gather only, 2=+softmax, 3=full agg
REPEAT = 1   # bench: run the whole body this many times inside one NEFF
LOCAL1 = False  # debug: single-core, collective replaced by local DMA copy


def _build_module():
    import concourse.bacc as bacc
    import concourse.tile as tile
    import concourse.mybir as mybir

    dt = mybir.dt
    Alu = mybir.AluOpType
    Act = mybir.ActivationFunctionType
    f16, f32, i16 = dt.float16, dt.float32, dt.int16

    nc = bacc.Bacc(
        "TRN2", target_bir_lowering=False, debug=False,
        num_devices=(1 if LOCAL1 else NCORES),
    )

    nbr_d = nc.dram_tensor("nbr", [MB, K], f32, kind="ExternalInput")
    node_d = nc.dram_tensor("node", [NB, K], f32, kind="ExternalInput")
    w_d = nc.dram_tensor("w", [O, K], f32, kind="ExternalInput")
    a1w_d = nc.dram_tensor("a1w", [128, O // 128], f32, kind="ExternalInput")
    a2_d = nc.dram_tensor("a2", [1, O], f32, kind="ExternalInput")
    gidx_d = nc.dram_tensor("gidx", [128, NT * IDXW], i16, kind="ExternalInput")
    dup_d = nc.dram_tensor("dup", [128, NT * S], f32, kind="ExternalInput")
    id_d = nc.dram_tensor("ident", [128, 128], f16, kind="ExternalInput")
    out_d = nc.dram_tensor("out", [NB, O], f32, kind="ExternalOutput")

    def _trace(tc):
        for _rep in range(REPEAT):
            _trace_once(tc)

    def _trace_once(tc):
        with (
            tc.tile_pool(name="cst", bufs=1) as cst,
            tc.tile_pool(name="dram", bufs=1, space="DRAM") as drp,
        ):
            # ---- constants (all contiguous / large-descriptor DMAs) --------
            ident = cst.tile([128, 128], f16)
            nc.sync.dma_start(ident[:], id_d[:])
            gsb = cst.tile([128, NT * IDXW], i16)
            nc.sync.dma_start(gsb[:], gidx_d[:])
            dsb = cst.tile([128, NT * S], f32)
            nc.sync.dma_start(dsb[:], dup_d[:])
            a2b = cst.tile([128, O], f32)
            nc.sync.dma_start(
                a2b[:], a2_d[0, :].partition_broadcast(128)
            )
            a1sb = cst.tile([128, O // 128], f16)
            nc.gpsimd.dma_start(a1sb[:], a1w_d[:])  # cast f32->f16, contiguous

            s1sb = cst.tile([128, NT], f32)
            v1row = cst.tile([1, K], f16)
            v1b = cst.tile([128, K], f16)

            with (
                tc.tile_pool(name="wt", bufs=1) as wtp,
                tc.tile_pool(name="augp", bufs=1) as augp,
                tc.tile_pool(name="nbp", bufs=2) as nbp,
                tc.tile_pool(name="ntp", bufs=3) as ntp,
            ):
                # wt[kp, kc, o] = W[o, kc*128 + kp]
                wt = wtp.tile([128, K // 128, O], f16)

                def _cast_nbr(i):
                    nbr16 = nbp.tile([128, K], f16, name="nbr16")
                    nc.gpsimd.dma_start(
                        nbr16[:], nbr_d[128 * i : 128 * (i + 1), :]
                    )
                    return nbr16

                def _transp_nbr(nbr16):
                    nbrT = ntp.tile([128, K // 128, 128], f16, name="nbrT")
                    nc.sync.dma_start_transpose(nbrT[:], nbr16[:])
                    return nbrT

                _c, _t = {}, {}
                with tc.tile_pool(name="w16p", bufs=1) as w16p:
                    # W casts in two 8MB halves; nbr0/1 casts interleaved
                    w16a = w16p.tile([128, O // 128, K], f16)
                    half = O // 256  # 4 oc per half
                    for h in range(2):
                        nc.gpsimd.dma_start(
                            w16a[:, half * h : half * (h + 1), :],
                            w_d[512 * h : 512 * (h + 1), :].rearrange(
                                "(c p) k -> p c k", p=128
                            ),
                        )
                        if STAGE >= 2 and h == 0:
                            _c[0] = _cast_nbr(0)
                            _c[1] = _cast_nbr(1)
                    # batched transposes: W first half, nbr0/1, W second half
                    for oc in range(half):
                        nc.sync.dma_start_transpose(
                            wt[:, :, 128 * oc : 128 * (oc + 1)], w16a[:, oc, :]
                        )
                    if STAGE >= 2:
                        _t[0] = _transp_nbr(_c.pop(0))
                        _t[1] = _transp_nbr(_c.pop(1))
                    for oc in range(half, O // 128):
                        nc.sync.dma_start_transpose(
                            wt[:, :, 128 * oc : 128 * (oc + 1)], w16a[:, oc, :]
                        )

                    # v1 = W.T @ a1 (per 512-chunk, 1 PSUM bank, ACT copies)
                    with tc.tile_pool(name="psv", bufs=2, space="PSUM") as psvp:
                        for fc in range(K // 512):
                            pv = psvp.tile([128, 512], f32, name="pv")
                            for oc in range(O // 128):
                                nc.tensor.matmul(
                                    pv[0:1, :],
                                    a1sb[:, oc : oc + 1],
                                    w16a[:, oc, 512 * fc : 512 * (fc + 1)],
                                    start=(oc == 0),
                                    stop=(oc == O // 128 - 1),
                                )
                            nc.scalar.copy(
                                v1row[0:1, 512 * fc : 512 * (fc + 1)], pv[0:1, :]
                            )
                    v1d = drp.tile([1, K], f16)
                    nc.sync.dma_start(v1d[:], v1row[:])
                    nc.sync.dma_start(v1b[:], v1d[0, :].partition_broadcast(128))

                # ---- main matmul: aug rows = [s2 | nbr @ W.T] --------------
                if STAGE < 2:
                    return
                aug = augp.tile([128, IT, ROW], f16)
                aug32 = aug.bitcast(f32)
                shard = drp.tile([MB, ROW], f16)
                if STAGE >= 3:
                    aug_full = drp.tile([M, ROW], f16)
                with (
                    tc.tile_pool(name="p2p", bufs=2) as p2p,
                    tc.tile_pool(name="s2p", bufs=2) as s2p,
                    tc.tile_pool(name="ndp", bufs=1) as ndp,
                    tc.tile_pool(name="psi", bufs=2, space="PSUM") as psip,
                ):
                    for i in range(IT):
                        nbrT = _t.pop(i) if i in _t else _transp_nbr(
                            _c.pop(i) if i in _c else _cast_nbr(i)
                        )
                        if i + 2 < IT and (i + 2) not in _c:
                            _c[i + 2] = _cast_nbr(i + 2)  # prefetch cast
                        ps = psip.tile([128, O], f32, name="ps")
                        for kc in range(K // 128):
                            nc.tensor.matmul(
                                ps[:, 0:512],
                                nbrT[:, kc, :],
                                wt[:, kc, 0:512],
                                start=(kc == 0),
                                stop=(kc == K // 128 - 1),
                            )
                            nc.tensor.matmul(
                                ps[:, 512:1024],
                                nbrT[:, kc, :],
                                wt[:, kc, 512:1024],
                                start=(kc == 0),
                                stop=(kc == K // 128 - 1),
                            )
                        # s2 = (neigh_t * a2).sum(free): DVE mult, ACT row-sum
                        prod2 = p2p.tile([128, O], f32, name="prod2")
                        s2c = s2p.tile([128, 1], f32, name="s2c")
                        nc.vector.tensor_tensor(
                            prod2[:], ps[:], a2b[:], Alu.mult
                        )
                        nc.scalar.activation(
                            prod2[:], prod2[:], Act.Copy, accum_out=s2c[:]
                        )
                        nc.scalar.copy(aug[:, i, ROFF : ROFF + O], ps[:])
                        nc.vector.tensor_copy(aug32[:, i, 0:1], s2c[:])
                        nc.sync.dma_start(
                            shard[128 * i : 128 * (i + 1), :], aug[:, i, :]
                        )
                        # ship each quarter as soon as it is complete
                        if STAGE >= 3 and (i + 1) % (IT // 4) == 0:
                            h = (i + 1) // (IT // 4) - 1
                            sl = shard[256 * h : 256 * (h + 1), :]
                            if LOCAL1:
                                nc.sync.dma_start(
                                    aug_full[2048 * h : 2048 * h + 256, :], sl
                                )
                            else:
                                nc.gpsimd.collective_compute(
                                    "AllGather",
                                    Alu.bypass,
                                    replica_groups=[list(range(NCORES))],
                                    ins=[sl.opt()],
                                    outs=[
                                        aug_full[
                                            2048 * h : 2048 * (h + 1), :
                                        ].opt()
                                    ],
                                )

                    # ---- s1 = node @ v1: one cast, DVE mult, ACT row-sum ---
                    node16 = ndp.tile([128, NT, K], f16, name="node16")
                    nc.gpsimd.dma_start(
                        node16[:], node_d.rearrange("(t p) k -> p t k", p=128)
                    )
                    for t in range(NT):
                        nc.vector.tensor_tensor(
                            node16[:, t, :], node16[:, t, :], v1b[:], Alu.mult
                        )
                        nc.scalar.activation(
                            node16[:, t, :], node16[:, t, :], Act.Copy,
                            accum_out=s1sb[:, t : t + 1],
                        )

            if STAGE < 3:
                return

            # ---- gather + softmax + aggregation ----------------------------
            if STAGE < 4:
                return
            with (
                tc.tile_pool(name="gp", bufs=3) as gp,
                tc.tile_pool(name="smp", bufs=2) as smp,
                tc.tile_pool(name="dkp", bufs=2) as dkp,
                tc.tile_pool(name="obp", bufs=2) as obp,
                tc.tile_pool(name="pso", bufs=2, space="PSUM") as psop,
            ):
                for t in range(NT):
                    G = gp.tile([128, S, ROW], f16, name="G")
                    for hh in range(2):
                        nc.gpsimd.dma_gather(
                            G[:, 5 * hh : 5 * (hh + 1), :],
                            aug_full[:],
                            gsb[:, IDXW * t + 40 * hh : IDXW * t + 40 * (hh + 1)],
                            640,
                            640,
                            ROW,
                        )
                    G32 = G.bitcast(f32)
                    s2g = smp.tile([128, S], f32, name="s2g")
                    nc.vector.tensor_copy(s2g[:], G32[:, :, 0:1])
                    # scores: B = s2g + dup + s1 ; leaky = max(B, 0.2B)
                    A_ = smp.tile([128, S], f32, name="A_")
                    nc.vector.tensor_tensor(
                        A_[:], s2g[:], dsb[:, S * t : S * (t + 1)], Alu.add
                    )
                    B_ = smp.tile([128, S], f32, name="B_")
                    nc.vector.tensor_scalar(
                        B_[:], A_[:], s1sb[:, t : t + 1], None, Alu.add
                    )
                    C_ = smp.tile([128, S], f32, name="C_")
                    nc.vector.tensor_scalar(C_[:], B_[:], 0.2, None, Alu.mult)
                    sc = smp.tile([128, S], f32, name="sc")
                    nc.vector.tensor_tensor(sc[:], B_[:], C_[:], Alu.max)
                    nmx = smp.tile([128, 1], f32, name="nmx")
                    nc.vector.tensor_reduce(
                        nmx[:], sc[:], mybir.AxisListType.X, Alu.max, negate=True
                    )
                    e_ = smp.tile([128, S], f32, name="e_")
                    den = smp.tile([128, 1], f32, name="den")
                    nc.scalar.activation(
                        e_[:], sc[:], Act.Exp, bias=nmx[:], scale=1.0,
                        accum_out=den[:],
                    )
                    rden = smp.tile([128, 1], f32, name="rden")
                    nc.vector.reciprocal(rden[:], den[:])
                    wts = smp.tile([128, S], f32, name="wts")
                    nc.vector.tensor_scalar(wts[:], e_[:], rden[:], None, Alu.mult)
                    # out_tile = sum_k diag(wts[:,k]) @ G_k  (PSUM accumulate)
                    po = psop.tile([128, O], f32, name="po")
                    for k in range(S):
                        dk = dkp.tile([128, 128], f16, name="dk")
                        nc.vector.tensor_scalar(
                            dk[:], ident[:], wts[:, k : k + 1], None, Alu.mult
                        )
                        nc.tensor.matmul(
                            po[:, 0:512],
                            dk[:],
                            G[:, k, ROFF : ROFF + 512],
                            start=(k == 0),
                            stop=(k == S - 1),
                        )
                        nc.tensor.matmul(
                            po[:, 512:1024],
                            dk[:],
                            G[:, k, ROFF + 512 : ROFF + 1024],
                            start=(k == 0),
                            stop=(k == S - 1),
                        )
                    osb = obp.tile([128, O], f32, name="osb")
                    nc.scalar.copy(osb[:], po[:])
                    nc.sync.dma_start(out_d[128 * t : 128 * (t + 1), :], osb[:])

    with tile.TileContext(nc) as tc:
        _trace(tc)
    nc.compile()
    return nc


def _get_nc():
    if "nc" not in _CACHE:
        _CACHE["nc"] = _build_module()
    return _CACHE["nc"]


def _prep_core_inputs(c, node, nbr, w, a1w, a2, idx, ident):
    idx_c = idx[c * NB : (c + 1) * NB]  # [512, S] int32
    gidx = np.empty((128, NT * IDXW), np.int16)
    dup = np.zeros((128, NT * S), np.float32)
    for t in range(NT):
        blk = idx_c[t * 128 : (t + 1) * 128]  # [128, S]
        # aug_full laid out [4 chunks, 8 ranks, 256 rows] (chunked AllGather):
        # global row j -> ((j>>8)&3)*2048 + (j>>10)*256 + (j&255)
        blkg = ((blk >> 8) & 3) * 2048 + (blk >> 10) * 256 + (blk & 255)
        # flat[ch*128 + p] = blkg[p, ch]; idx_sb[p, s] = flat[s*16 + p%16]
        flat = blkg.T.reshape(-1).astype(np.int16)  # [1280]
        wrapped = flat.reshape(IDXW, 16)            # [80, 16]
        gidx[:, IDXW * t : IDXW * (t + 1)] = np.tile(wrapped.T, (8, 1))
        d = np.zeros((128, S), np.float32)
        for k in range(1, S):
            d[:, k] = np.where(
                (blk[:, :k] == blk[:, k : k + 1]).any(axis=1), -1e30, 0.0
            )
        dup[:, S * t : S * (t + 1)] = d
    return {
        "nbr": np.ascontiguousarray(nbr[c * MB : (c + 1) * MB]),
        "node": np.ascontiguousarray(node[c * NB : (c + 1) * NB]),
        "w": w,
        "a1w": a1w,
        "a2": a2,
        "gidx": gidx,
        "dup": dup,
        "ident": ident,
    }


def kernel(**inputs) -> np.ndarray:
    global LAST_RESULTS
    from concourse import bass_utils

    node = np.ascontiguousarray(np.asarray(inputs["node_features"], np.float32))
    nbr = np.ascontiguousarray(np.asarray(inputs["neighbor_features"], np.float32))
    w = np.ascontiguousarray(np.asarray(inputs["weight"], np.float32))
    att = np.asarray(inputs["attention_vector"], np.float32)
    idx = np.asarray(inputs["neighbor_idx"], np.int32)

    a1 = att[:O, 0]
    a2 = np.ascontiguousarray(att[O:, 0].reshape(1, O))
    # a1 pre-wrapped for the v1 matmul: a1w[p, oc] = a1[oc*128 + p]
    a1w = np.ascontiguousarray(a1.reshape(O // 128, 128).T)
    ident = np.eye(128, dtype=np.float16)

    in_maps = [
        _prep_core_inputs(c, node, nbr, w, a1w, a2, idx, ident)
        for c in range(NCORES)
    ]
    nc = _get_nc()
    res = bass_utils.run_bass_kernel_spmd(
        nc,
        in_maps,
        core_ids=list(range(NCORES)),
        trace=TRACE,
        **TRACE_KW,
    )
    LAST_RESULTS = res
    return np.concatenate([r["out"] for r in res.results], axis=0)


if __name__ == "__main__":
    rng = np.random.default_rng(0)
    ins = {
        "node_features": rng.standard_normal((N, K), dtype=np.float32),
        "neighbor_features": rng.standard_normal((M, K), dtype=np.float32),
        "weight": rng.standard_normal((O, K), dtype=np.float32) / np.sqrt(K),
        "attention_vector": (rng.standard_normal((2 * O, 1)) * 0.05).astype(
            np.float32
        ),
        "neighbor_idx": rng.integers(0, M, (N, S), dtype=np.int32),
    }
    out = kernel(**ins)
    print("out", out.shape, out.dtype, np.abs(out).max())
